# revision 18
# baseline (speedup 1.0000x reference)
"""CARAFE content-aware upsampling (scale=2, K=5, encoder 3x3) on 8 TRN2 NeuronCores.

Sharding: 8 shards = batch(4) x H-halves(2), pure data parallel (1-row halo
per shard handled host-side). Channel-major fp16 pipeline, fully pipelined at
(row-half x subgrid) granularity:

  1. compress 1x1 conv      : PE matmul, stationary widened [C,2*CC] so PSUM
                              holds TWO copies of k1 (partitions 64-127 are a
                              row-shifted copy, enabling encoder tap pairing)
  2. encoder 3x3 conv       : 6 accumulating PE matmuls per 8-row block
                              (3 tap-PAIRS at K=128 + 3 singles at K=64)
  3. e = exp(enc + b)       : ACT
  4. combined masses        : one PE matmul [100->40] = 36 shifted-tap masses
                              (dx-major order) + 4 softmax denominators S
  5. r = exp(-ln S)         : ACT; R9 one-hot PE matmul expands r to 36 rows;
                              DVE multiplies masses in place (normalization)
  6. mask broadcast         : per (row-half, subgrid): bounce to DRAM, then a
                              stride-0 SWDGE DMA replicates [9,16,64] masses
                              to all 128 partitions
  7. reassembly             : 3 DVE multiplies per chunk (dy-triples share one
                              overlapping-window AP); 9-tap sum done by ONE
                              PE matmul per 512-px block using a stride-0
                              PSUM out-AP (in-instruction accumulation)
  8. out = fp16 staging     : ACT PSUM->SBUF interleaved store, one contig
                              2MB-class DMA per row-half; host casts to fp32
"""

import numpy as np

SCALE, KK, EK = 2, 5, 3
B, C, H, W = 4, 128, 64, 64
CC, KC = 64, 100
HS = H // 2          # 32 interior rows per shard
PIX = HS * W
NCORES = 8

# taps in dx-major order: tap index t = (dx+1)*3 + (dy+1)
TAPS = [(dy, dx) for dx in (-1, 0, 1) for dy in (-1, 0, 1)]

# accumulate the 9 tap products with a single stride-0 matmul per 512-px block
# (rejected by the MATMULT ISA encoder: out APs cannot repeat addresses)
ACC_FUSED = False

_PROGRAM = None


def _build_A():
    """[100, 40] combine matrix: cols 0-35 = shifted-tap masses (dx-major
    within each subgrid), cols 36-39 = softmax denominators per subgrid."""
    A = np.zeros((KC, 40), dtype=np.float32)
    for r1 in range(2):
        for r2 in range(2):
            q = 2 * r1 + r2
            for i in range(KK):
                for j in range(KK):
                    dy = (r1 + i - 2) // 2
                    dx = (r2 + j - 2) // 2
                    tidx = (dx + 1) * 3 + (dy + 1)
                    A[4 * (5 * i + j) + q, q * 9 + tidx] += 1.0
            A[np.arange(q, KC, 4), 36 + q] = 1.0
    return A


def _build_program():
    import concourse.bass as bass
    import concourse.tile as tile
    from concourse.tile import add_dep_helper
    from concourse import bacc, mybir

    f32 = mybir.dt.float32
    f16 = mybir.dt.float16
    AF = mybir.ActivationFunctionType

    nc = bacc.Bacc("TRN2", target_bir_lowering=False, debug=False,
                   num_devices=NCORES)

    xin = nc.declare_dram_parameter("xs", [C, HS + 2, W], f32, isOutput=False)
    cw2 = nc.declare_dram_parameter("comp_w2", [C, 2 * CC], f16, isOutput=False)
    ewp = nc.declare_dram_parameter("enc_wp", [2 * CC, 3, KC], f16, isOutput=False)
    ews = nc.declare_dram_parameter("enc_ws", [CC, 3, KC], f16, isOutput=False)
    eb = nc.declare_dram_parameter("enc_b", [KC, 1], f32, isOutput=False)
    out = nc.declare_dram_parameter("out", [C, 2 * HS, 2 * W], f16, isOutput=True)

    # pad combine matrix to 68 outputs: masses at PSUM partitions 0-35,
    # denominators at 64-67 (PSUM reads must start at a 0/32/64/96 partition)
    A40 = _build_A()
    A68 = np.zeros((KC, 68), dtype=np.float16)
    A68[:, 0:36] = A40[:, 0:36]
    A68[:, 64:68] = A40[:, 36:40]
    A_dram = nc.inline_tensor(A68, name="A_cmb")
    R9 = np.zeros((4, 36), dtype=np.float16)
    for qq in range(4):
        R9[qq, qq * 9 : (qq + 1) * 9] = 1.0
    R9_dram = nc.inline_tensor(R9, name="R9")
    I_dram = nc.inline_tensor(np.eye(128, dtype=np.float16), name="ident")

    # masks bounced per row-half: [rh][q][tap][16][64] fp16
    mu_dram = nc.dram_tensor("mu_bounce", [2, 4, 9, 16, W], f16)

    with tile.TileContext(nc) as tc:
        with (
            tc.tile_pool(name="singles", bufs=1) as singles,
            tc.tile_pool(name="work", bufs=2) as work,
            tc.tile_pool(name="mc", bufs=3) as mc,
            tc.tile_pool(name="tp", bufs=3) as tp,
            tc.tile_pool(name="ps1", bufs=2, space="PSUM") as ps1,
            tc.tile_pool(name="pse", bufs=2, space="PSUM") as pse,
            tc.tile_pool(name="psc", bufs=1, space="PSUM") as psc,
            tc.tile_pool(name="psr", bufs=1, space="PSUM") as psr,
            tc.tile_pool(name="psa", bufs=2, space="PSUM") as psa,
        ):
            # ---------------- persistent SBUF ----------------
            x16 = [singles.tile([C, HS + 2, W], f16, tag=f"x16_{d}",
                                name=f"x16_{d}")
                   for d in range(3)]  # dx = -1, 0, +1 pre-shifted copies
            # two k1 copies: partitions 0-63 = k1, 64-127 = k1 shifted 1 row up
            k1two = singles.tile([C, HS + 2, W + 2], f16, tag="k1two")
            e_sb = singles.tile([KC, HS, W], f16, tag="e_sb")
            mu16 = singles.tile([36, HS, W], f16, tag="mu16")
            out16 = [singles.tile([C, 16, 2, W, 2], f16, tag=f"o16_{rh}",
                                  name=f"o16_{rh}")
                     for rh in range(2)]
            cw2_sb = singles.tile([C, 2 * CC], f16, tag="cw2")
            ewp_sb = singles.tile([2 * CC, 3, KC], f16, tag="ewp")
            ews_sb = singles.tile([CC, 3, KC], f16, tag="ews")
            eb_sb = singles.tile([KC, 1], f32, tag="eb")
            A_sb = singles.tile([KC, 68], f16, tag="A_sb")
            R9_sb = singles.tile([4, 36], f16, tag="R9_sb")
            id_sb = singles.tile([128, 128], f16, tag="id_sb")

            # ---------------- loads ----------------
            # x load with fp32 -> fp16 cast (SWDGE), 4 splits for fast ramp
            for s, (a, b) in enumerate([(0, 9), (9, 17), (17, 25), (25, 34)]):
                nc.gpsimd.dma_start(out=x16[1][:, a:b, :], in_=xin[:, a:b, :])
            nc.sync.dma_start(out=cw2_sb, in_=cw2[:])
            nc.sync.dma_start(out=ewp_sb, in_=ewp[:])
            nc.sync.dma_start(out=ews_sb, in_=ews[:])
            nc.sync.dma_start(out=eb_sb, in_=eb[:])
            nc.sync.dma_start(out=A_sb, in_=A_dram[:])
            nc.sync.dma_start(out=R9_sb, in_=R9_dram[:])
            nc.sync.dma_start(out=id_sb, in_=I_dram[:])

            nc.vector.memset(x16[0][:, :, 0:1], 0.0)
            nc.vector.memset(x16[2][:, :, W - 1 : W], 0.0)
            nc.vector.memset(k1two[:, :, 0:1], 0.0)
            nc.vector.memset(k1two[:, :, W + 1 : W + 2], 0.0)
            nc.vector.memset(k1two[64:128, HS + 1 : HS + 2, :], 0.0)
            nc.vector.tensor_copy(x16[0][:, :, 1:W], x16[1][:, :, 0 : W - 1])
            nc.vector.tensor_copy(x16[2][:, :, 0 : W - 1], x16[1][:, :, 1:W])

            # ---------------- stage 1: compress conv ----------------
            # emits the 1x1 conv for k1two rows [r0, r1); PSUM partitions
            # 64-127 hold an identical copy that lands one row higher.
            def emit_stage1(r0, r1, on_act=False):
                # comp_b is folded into the encoder bias host-side, so the
                # PSUM -> SBUF move is a plain cast copy. Early blocks use
                # DVE (idle then, and ACT table loads gate the mask chain).
                ps = ps1.tile([C, r1 - r0, W], f32, tag="ps1",
                              name=f"s1_{r0}")
                nc.tensor.matmul(ps, cw2_sb, x16[1][:, r0:r1, :],
                                 start=True, stop=True)
                cp = nc.scalar.copy if on_act else nc.vector.tensor_copy
                cp(k1two[0:64, r0:r1, 1 : 1 + W], ps[0:64])
                s0 = 1 if r0 == 0 else 0
                cp(k1two[64:128, r0 + s0 - 1 : r1 - 1, 1 : 1 + W],
                   ps[64:128, s0 : r1 - r0])

            # ---------------- stages 2-5 for one 8-row block ----------------
            def emit_mask_block(blk):
                y0 = 8 * blk
                ps = pse.tile([KC, 8, W], f32, tag="pse", name=f"enc_{y0}")
                # 3 tap-pairs (di=0&1 via the shifted copy) + 3 singles (di=2)
                for j in range(3):
                    nc.tensor.matmul(ps, ewp_sb[:, j, :],
                                     k1two[:, y0 : y0 + 8, j : j + W],
                                     start=(j == 0), stop=False)
                for j in range(3):
                    nc.tensor.matmul(ps, ews_sb[:, j, :],
                                     k1two[0:64, y0 + 2 : y0 + 10, j : j + W],
                                     start=False, stop=(j == 2))
                nc.scalar.activation(e_sb[:, y0 : y0 + 8, :], ps, AF.Exp,
                                     bias=eb_sb, scale=1.0)
                # combine: 36 masses + 4 denominators in one matmul
                pc = psc.tile([68, 8, W], f32, tag="psc", name=f"cmb_{y0}")
                nc.tensor.matmul(pc, A_sb, e_sb[:, y0 : y0 + 8, :],
                                 start=True, stop=True)
                r16 = work.tile([4, 8, W], f16, tag="r16", name=f"r16_{y0}")
                m36 = work.tile([36, 8, W], f16, tag="m36", name=f"m36_{y0}")
                with nc.allow_low_precision("softmax normalizer to fp16"):
                    nc.vector.reciprocal(r16, pc[64:68])
                nc.scalar.copy(m36, pc[0:36])
                pr = psr.tile([36, 8, W], f32, tag="psr", name=f"r36_{y0}")
                nc.tensor.matmul(pr, R9_sb, r16, start=True, stop=True)
                norm_ops[blk] = (m36, pr)

            # the normalize multiply is emitted separately so the DVE queue
            # can run row-half-0 products before row-half-1 norms
            norm_ops = {}

            def emit_norm(blk):
                y0 = 8 * blk
                m36, pr = norm_ops[blk]
                nc.vector.tensor_mul(mu16[:, y0 : y0 + 8, :], m36, pr)

            bounce = [None, None]

            def emit_bounce(rh):
                dst = bass.AP(tensor=mu_dram, offset=rh * 36 * 16 * W,
                              ap=[[16 * W, 36], [1, 16 * W]])
                bounce[rh] = nc.sync.dma_start(
                    out=dst, in_=mu16[:, 16 * rh : 16 * rh + 16, :])

            # ---------------- reassembly chunk (rh, q) ----------------
            def emit_chunk(rh, q):
                r1, r2 = q >> 1, q & 1
                mcast = mc.tile([128, 9, 16, W], f16, tag="mcast",
                                name=f"mc_{rh}_{q}")
                mflat = mcast.rearrange("p t h w -> p (t h w)")
                src = bass.AP(tensor=mu_dram,
                              offset=(rh * 4 + q) * 9 * 16 * W,
                              ap=[[0, 128], [1, 9 * 16 * W]])
                bc = nc.gpsimd.dma_start(out=mflat, in_=src)
                add_dep_helper(bc.ins, bounce[rh].ins, sync=True,
                               reason="mask broadcast after bounce")

                tmp = tp.tile([128, 9, 16, W], f16, tag="tmp",
                              name=f"tmp_{rh}_{q}")
                # products: one DVE op per dx (3 dy-taps share an
                # overlapping-row window AP)
                for dxi in range(3):
                    basep = x16[dxi][:, 16 * rh : 16 * rh + 16, :]
                    pdim = [list(p) for p in basep.ap][0]
                    in0 = bass.AP(tensor=basep.tensor, offset=basep.offset,
                                  ap=[pdim, [W, 3], [W, 16], [1, W]])
                    nc.vector.tensor_mul(tmp[:, 3 * dxi : 3 * dxi + 3],
                                         in0, mcast[:, 3 * dxi : 3 * dxi + 3])
                tflat = tmp.rearrange("p t h w -> p t (h w)")
                for b in range(2):
                    acc = psa.tile([C, 8, W], f32, tag="acc",
                                   name=f"acc_{rh}_{q}_{b}")
                    if ACC_FUSED:
                        # moving tile caps at 128x4096: 8 taps fused + 1
                        mov = tflat[:, 0:8, 512 * b : 512 * (b + 1)]
                        oap = acc.rearrange("p h w -> p (h w)")
                        oap8 = oap.unsqueeze(1).broadcast_to([C, 8, 512])
                        nc.tensor.matmul(oap8, id_sb, mov,
                                         start=True, stop=False,
                                         skip_group_check=True)
                        nc.tensor.matmul(oap, id_sb,
                                         tflat[:, 8, 512 * b : 512 * (b + 1)],
                                         start=False, stop=True,
                                         skip_group_check=True)
                    else:
                        for t in range(9):
                            nc.tensor.matmul(
                                acc.rearrange("p h w -> p (h w)"), id_sb,
                                tflat[:, t, 512 * b : 512 * (b + 1)],
                                start=(t == 0), stop=(t == 8),
                                skip_group_check=True)
                    nc.scalar.copy(
                        out16[rh][:, 8 * b : 8 * b + 8, r1, :, r2], acc)

            def emit_store(rh):
                nc.sync.dma_start(out=out[:, 32 * rh : 32 * rh + 32, :],
                                  in_=out16[rh])

            # ---------------- emission schedule ----------------
            # PE queue runs every mask matmul before the reassembly
            # accumulations; DVE queue runs row-half-0 products before
            # row-half-1 norms, so neither engine stalls on the other.
            emit_stage1(0, 8)
            emit_stage1(8, 16)
            emit_mask_block(0)
            emit_stage1(16, 24)
            emit_mask_block(1)
            emit_norm(0)
            emit_norm(1)
            emit_bounce(0)
            # row-half 1 mask matmuls fill PE while broadcasts stream
            emit_stage1(24, 32, on_act=True)
            emit_stage1(32, 34, on_act=True)
            emit_mask_block(2)
            emit_mask_block(3)
            emit_chunk(0, 0)
            emit_chunk(0, 1)
            emit_norm(2)
            emit_norm(3)
            emit_bounce(1)
            # late stage-1 copies ride ACT (off the bounce critical path)
            emit_chunk(0, 2)
            emit_chunk(0, 3)
            emit_store(0)
            for q in range(4):
                emit_chunk(1, q)
            emit_store(1)

    nc.compile()
    return nc


def _get_program():
    global _PROGRAM
    if _PROGRAM is None:
        _PROGRAM = _build_program()
    return _PROGRAM


def _shard_inputs(x, comp_w, comp_b, enc_w, enc_b):
    comp_wT = comp_w[:, :, 0, 0].T.astype(np.float16)          # [C, CC]
    comp_w2 = np.ascontiguousarray(
        np.concatenate([comp_wT, comp_wT], axis=1))            # [C, 2CC]
    # enc_w [KC, CC, 3, 3] -> tap-pair stationaries
    ew = enc_w.astype(np.float16)
    enc_wp = np.zeros((2 * CC, 3, KC), dtype=np.float16)
    enc_ws = np.zeros((CC, 3, KC), dtype=np.float16)
    for j in range(3):
        enc_wp[0:CC, j, :] = ew[:, :, 0, j].T      # di = 0 (bottom copy)
        enc_wp[CC:, j, :] = ew[:, :, 1, j].T       # di = 1 (shifted copy)
        enc_ws[:, j, :] = ew[:, :, 2, j].T         # di = 2 (single)
    # fold comp_b through the encoder taps into the encoder bias
    eb_eff = (enc_b.astype(np.float64)
              + enc_w.astype(np.float64).sum(axis=(2, 3))
              @ comp_b.astype(np.float64))
    ebv = np.ascontiguousarray(eb_eff.astype(np.float32).reshape(KC, 1))
    in_maps = []
    for core in range(NCORES):
        b, h = divmod(core, 2)
        xs = np.zeros((C, HS + 2, W), dtype=np.float32)
        lo = h * HS - 1
        s0, s1 = max(0, lo), min(H, lo + HS + 2)
        xs[:, s0 - lo : s1 - lo, :] = x[b, :, s0:s1, :]
        in_maps.append({
            "xs": np.ascontiguousarray(xs),
            "comp_w2": comp_w2,
            "enc_wp": np.ascontiguousarray(enc_wp),
            "enc_ws": np.ascontiguousarray(enc_ws),
            "enc_b": ebv,
        })
    return in_maps


def _run(inputs, trace=False):
    from concourse.bass_utils import run_bass_kernel_spmd

    nc = _get_program()
    in_maps = _shard_inputs(**inputs)
    res = run_bass_kernel_spmd(nc, in_maps, list(range(NCORES)), trace=trace)
    out = np.empty((B, C, 2 * H, 2 * W), dtype=np.float32)
    for core in range(NCORES):
        b, h = divmod(core, 2)
        out[b, :, h * 2 * HS : (h + 1) * 2 * HS, :] = \
            res.results[core]["out"].astype(np.float32)
    return out, res.exec_time_ns


def kernel(x, comp_w, comp_b, enc_w, enc_b):
    out, _ = _run(dict(x=np.asarray(x), comp_w=np.asarray(comp_w),
                       comp_b=np.asarray(comp_b), enc_w=np.asarray(enc_w),
                       enc_b=np.asarray(enc_b)))
    return out


# revision 23
# speedup vs baseline: 1.0421x; 1.0421x over previous
"""CARAFE content-aware upsampling (scale=2, K=5, encoder 3x3) on 8 TRN2 NeuronCores.

Sharding: 8 shards = batch(4) x H-halves(2), pure data parallel (1-row halo
per shard handled host-side). Channel-major fp16 pipeline, fully pipelined at
(row-half x subgrid) granularity:

  1. compress 1x1 conv      : PE matmul, stationary widened [C,2*CC] so PSUM
                              holds TWO copies of k1 (partitions 64-127 are a
                              row-shifted copy, enabling encoder tap pairing)
  2. encoder 3x3 conv       : 6 accumulating PE matmuls per 8-row block
                              (3 tap-PAIRS at K=128 + 3 singles at K=64)
  3. e = exp(enc + b)       : ACT
  4. combined masses        : one PE matmul [100->40] = 36 shifted-tap masses
                              (dx-major order) + 4 softmax denominators S
  5. r = exp(-ln S)         : ACT; R9 one-hot PE matmul expands r to 36 rows;
                              DVE multiplies masses in place (normalization)
  6. mask broadcast         : per (row-half, subgrid): bounce to DRAM, then a
                              stride-0 SWDGE DMA replicates [9,16,64] masses
                              to all 128 partitions
  7. reassembly             : 3 DVE multiplies per chunk (dy-triples share one
                              overlapping-window AP); 9-tap sum done by ONE
                              PE matmul per 512-px block using a stride-0
                              PSUM out-AP (in-instruction accumulation)
  8. out = fp16 staging     : ACT PSUM->SBUF interleaved store, one contig
                              2MB-class DMA per row-half; host casts to fp32
"""

import numpy as np

SCALE, KK, EK = 2, 5, 3
B, C, H, W = 4, 128, 64, 64
CC, KC = 64, 100
HS = H // 2          # 32 interior rows per shard
PIX = HS * W
NCORES = 8

# taps in dx-major order: tap index t = (dx+1)*3 + (dy+1)
TAPS = [(dy, dx) for dx in (-1, 0, 1) for dy in (-1, 0, 1)]

# accumulate the 9 tap products with a single stride-0 matmul per 512-px block
# (rejected by the MATMULT ISA encoder: out APs cannot repeat addresses)
ACC_FUSED = False

_PROGRAM = None


def _build_A():
    """[100, 40] combine matrix: cols 0-35 = shifted-tap masses (dx-major
    within each subgrid), cols 36-39 = softmax denominators per subgrid."""
    A = np.zeros((KC, 40), dtype=np.float32)
    for r1 in range(2):
        for r2 in range(2):
            q = 2 * r1 + r2
            for i in range(KK):
                for j in range(KK):
                    dy = (r1 + i - 2) // 2
                    dx = (r2 + j - 2) // 2
                    tidx = (dx + 1) * 3 + (dy + 1)
                    A[4 * (5 * i + j) + q, q * 9 + tidx] += 1.0
            A[np.arange(q, KC, 4), 36 + q] = 1.0
    return A


def _build_program():
    import concourse.bass as bass
    import concourse.tile as tile
    from concourse.tile import add_dep_helper
    from concourse import bacc, mybir

    f32 = mybir.dt.float32
    f16 = mybir.dt.float16
    AF = mybir.ActivationFunctionType

    nc = bacc.Bacc("TRN2", target_bir_lowering=False, debug=False,
                   num_devices=NCORES)

    xin = nc.declare_dram_parameter("xs", [C, HS + 2, W], f32, isOutput=False)
    cw2 = nc.declare_dram_parameter("comp_w2", [C, 2 * CC], f16, isOutput=False)
    ewp = nc.declare_dram_parameter("enc_wp", [2 * CC, 3, KC], f16, isOutput=False)
    ews = nc.declare_dram_parameter("enc_ws", [CC, 3, KC], f16, isOutput=False)
    eb = nc.declare_dram_parameter("enc_b", [KC, 1], f32, isOutput=False)
    out = nc.declare_dram_parameter("out", [C, 2 * HS, 2 * W], f16, isOutput=True)

    # pad combine matrix to 68 outputs: masses at PSUM partitions 0-35,
    # denominators at 64-67 (PSUM reads must start at a 0/32/64/96 partition)
    A40 = _build_A()
    A68 = np.zeros((KC, 68), dtype=np.float16)
    A68[:, 0:36] = A40[:, 0:36]
    A68[:, 64:68] = A40[:, 36:40]
    A_dram = nc.inline_tensor(A68, name="A_cmb")
    R9 = np.zeros((4, 36), dtype=np.float16)
    for qq in range(4):
        R9[qq, qq * 9 : (qq + 1) * 9] = 1.0
    R9_dram = nc.inline_tensor(R9, name="R9")
    I_dram = nc.inline_tensor(np.eye(128, dtype=np.float16), name="ident")

    # masks bounced per row-half: [rh][q][tap][16][64] fp16
    mu_dram = nc.dram_tensor("mu_bounce", [2, 4, 9, 16, W], f16)

    with tile.TileContext(nc) as tc:
        with (
            tc.tile_pool(name="singles", bufs=1) as singles,
            tc.tile_pool(name="work", bufs=2) as work,
            tc.tile_pool(name="mc", bufs=3) as mc,
            tc.tile_pool(name="tp", bufs=3) as tp,
            tc.tile_pool(name="ps1", bufs=2, space="PSUM") as ps1,
            tc.tile_pool(name="pse", bufs=2, space="PSUM") as pse,
            tc.tile_pool(name="psc", bufs=1, space="PSUM") as psc,
            tc.tile_pool(name="psr", bufs=1, space="PSUM") as psr,
            tc.tile_pool(name="psa", bufs=2, space="PSUM") as psa,
        ):
            # ---------------- persistent SBUF ----------------
            x16 = [singles.tile([C, HS + 2, W], f16, tag=f"x16_{d}",
                                name=f"x16_{d}")
                   for d in range(3)]  # dx = -1, 0, +1 pre-shifted copies
            # two k1 copies: partitions 0-63 = k1, 64-127 = k1 shifted 1 row up
            k1two = singles.tile([C, HS + 2, W + 2], f16, tag="k1two")
            e_sb = singles.tile([KC, HS, W], f16, tag="e_sb")
            mu16 = singles.tile([36, HS, W], f16, tag="mu16")
            out16 = [singles.tile([C, 16, 2, W, 2], f16, tag=f"o16_{rh}",
                                  name=f"o16_{rh}")
                     for rh in range(2)]
            cw2_sb = singles.tile([C, 2 * CC], f16, tag="cw2")
            ewp_sb = singles.tile([2 * CC, 3, KC], f16, tag="ewp")
            ews_sb = singles.tile([CC, 3, KC], f16, tag="ews")
            eb_sb = singles.tile([KC, 1], f32, tag="eb")
            A_sb = singles.tile([KC, 68], f16, tag="A_sb")
            R9_sb = singles.tile([4, 36], f16, tag="R9_sb")
            id_sb = singles.tile([128, 128], f16, tag="id_sb")

            # ---------------- loads ----------------
            # x load with fp32 -> fp16 cast (SWDGE), 4 splits for fast ramp
            for s, (a, b) in enumerate([(0, 9), (9, 17), (17, 25), (25, 34)]):
                nc.gpsimd.dma_start(out=x16[1][:, a:b, :], in_=xin[:, a:b, :])
            nc.sync.dma_start(out=cw2_sb, in_=cw2[:])
            nc.sync.dma_start(out=ewp_sb, in_=ewp[:])
            nc.sync.dma_start(out=ews_sb, in_=ews[:])
            nc.sync.dma_start(out=eb_sb, in_=eb[:])
            nc.sync.dma_start(out=A_sb, in_=A_dram[:])
            nc.sync.dma_start(out=R9_sb, in_=R9_dram[:])
            nc.sync.dma_start(out=id_sb, in_=I_dram[:])

            # PE warm-up: ~4us of back-to-back junk matmuls while the x-load
            # streams, so the HAM clock gate reaches 8/8 (2.4 GHz) before the
            # real mask matmuls start instead of running them all at 1.2 GHz.
            warm = psa.tile([128, 8, W], f32, tag="acc", name="warmup")
            for _ in range(40):
                nc.tensor.matmul(warm[:, 0:2, :], id_sb, id_sb,
                                 start=True, stop=True, skip_group_check=True)

            nc.vector.memset(x16[0][:, :, 0:1], 0.0)
            nc.vector.memset(x16[2][:, :, W - 1 : W], 0.0)
            nc.vector.memset(k1two[:, :, 0:1], 0.0)
            nc.vector.memset(k1two[:, :, W + 1 : W + 2], 0.0)
            nc.vector.memset(k1two[64:128, HS + 1 : HS + 2, :], 0.0)
            nc.vector.tensor_copy(x16[0][:, :, 1:W], x16[1][:, :, 0 : W - 1])
            nc.vector.tensor_copy(x16[2][:, :, 0 : W - 1], x16[1][:, :, 1:W])

            # ---------------- stage 1: compress conv ----------------
            # emits the 1x1 conv for k1two rows [r0, r1); PSUM partitions
            # 64-127 hold an identical copy that lands one row higher.
            def emit_stage1(r0, r1, on_act=False):
                # comp_b is folded into the encoder bias host-side, so the
                # PSUM -> SBUF move is a plain cast copy. Early blocks use
                # DVE (idle then, and ACT table loads gate the mask chain).
                ps = ps1.tile([C, r1 - r0, W], f32, tag="ps1",
                              name=f"s1_{r0}")
                nc.tensor.matmul(ps, cw2_sb, x16[1][:, r0:r1, :],
                                 start=True, stop=True)
                cp = nc.scalar.copy if on_act else nc.vector.tensor_copy
                cp(k1two[0:64, r0:r1, 1 : 1 + W], ps[0:64])
                s0 = 1 if r0 == 0 else 0
                cp(k1two[64:128, r0 + s0 - 1 : r1 - 1, 1 : 1 + W],
                   ps[64:128, s0 : r1 - r0])

            # ---------------- stages 2-5 for one 8-row block ----------------
            def emit_mask_block(blk):
                y0 = 8 * blk
                ps = pse.tile([KC, 8, W], f32, tag="pse", name=f"enc_{y0}")
                # 3 tap-pairs (di=0&1 via the shifted copy) + 3 singles (di=2)
                for j in range(3):
                    nc.tensor.matmul(ps, ewp_sb[:, j, :],
                                     k1two[:, y0 : y0 + 8, j : j + W],
                                     start=(j == 0), stop=False)
                for j in range(3):
                    nc.tensor.matmul(ps, ews_sb[:, j, :],
                                     k1two[0:64, y0 + 2 : y0 + 10, j : j + W],
                                     start=False, stop=(j == 2))
                nc.scalar.activation(e_sb[:, y0 : y0 + 8, :], ps, AF.Exp,
                                     bias=eb_sb, scale=1.0)
                # combine: 36 masses + 4 denominators in one matmul
                pc = psc.tile([68, 8, W], f32, tag="psc", name=f"cmb_{y0}")
                nc.tensor.matmul(pc, A_sb, e_sb[:, y0 : y0 + 8, :],
                                 start=True, stop=True)
                s32 = work.tile([4, 8, W], f32, tag="s32", name=f"s32_{y0}")
                r32 = work.tile([4, 8, W], f32, tag="r32", name=f"r32_{y0}")
                r16 = work.tile([4, 8, W], f16, tag="r16", name=f"r16_{y0}")
                m36 = work.tile([36, 8, W], f16, tag="m36", name=f"m36_{y0}")
                # the bitwise-seed reciprocal cannot read PSUM; stage via ACT
                nc.scalar.copy(s32, pc[64:68])
                nc.vector.reciprocal_approx_fast(r32, s32)
                nc.vector.tensor_copy(r16, r32)
                nc.scalar.copy(m36, pc[0:36])
                pr = psr.tile([36, 8, W], f32, tag="psr", name=f"r36_{y0}")
                nc.tensor.matmul(pr, R9_sb, r16, start=True, stop=True)
                norm_ops[blk] = (m36, pr)

            # the normalize multiply is emitted separately so the DVE queue
            # can run row-half-0 products before row-half-1 norms
            norm_ops = {}

            def emit_norm(blk):
                y0 = 8 * blk
                m36, pr = norm_ops[blk]
                nc.vector.tensor_mul(mu16[:, y0 : y0 + 8, :], m36, pr)

            bounce = [None, None]

            def emit_bounce(rh):
                dst = bass.AP(tensor=mu_dram, offset=rh * 36 * 16 * W,
                              ap=[[16 * W, 36], [1, 16 * W]])
                bounce[rh] = nc.sync.dma_start(
                    out=dst, in_=mu16[:, 16 * rh : 16 * rh + 16, :])

            # ---------------- reassembly chunk (rh, q) ----------------
            def emit_chunk(rh, q):
                r1, r2 = q >> 1, q & 1
                mcast = mc.tile([128, 9, 16, W], f16, tag="mcast",
                                name=f"mc_{rh}_{q}")
                mflat = mcast.rearrange("p t h w -> p (t h w)")
                src = bass.AP(tensor=mu_dram,
                              offset=(rh * 4 + q) * 9 * 16 * W,
                              ap=[[0, 128], [1, 9 * 16 * W]])
                bc = nc.gpsimd.dma_start(out=mflat, in_=src)
                add_dep_helper(bc.ins, bounce[rh].ins, sync=True,
                               reason="mask broadcast after bounce")

                tmp = tp.tile([128, 9, 16, W], f16, tag="tmp",
                              name=f"tmp_{rh}_{q}")
                # products: one DVE op per dx (3 dy-taps share an
                # overlapping-row window AP)
                for dxi in range(3):
                    basep = x16[dxi][:, 16 * rh : 16 * rh + 16, :]
                    pdim = [list(p) for p in basep.ap][0]
                    in0 = bass.AP(tensor=basep.tensor, offset=basep.offset,
                                  ap=[pdim, [W, 3], [W, 16], [1, W]])
                    nc.vector.tensor_mul(tmp[:, 3 * dxi : 3 * dxi + 3],
                                         in0, mcast[:, 3 * dxi : 3 * dxi + 3])
                tflat = tmp.rearrange("p t h w -> p t (h w)")
                for b in range(2):
                    acc = psa.tile([C, 8, W], f32, tag="acc",
                                   name=f"acc_{rh}_{q}_{b}")
                    if ACC_FUSED:
                        # moving tile caps at 128x4096: 8 taps fused + 1
                        mov = tflat[:, 0:8, 512 * b : 512 * (b + 1)]
                        oap = acc.rearrange("p h w -> p (h w)")
                        oap8 = oap.unsqueeze(1).broadcast_to([C, 8, 512])
                        nc.tensor.matmul(oap8, id_sb, mov,
                                         start=True, stop=False,
                                         skip_group_check=True)
                        nc.tensor.matmul(oap, id_sb,
                                         tflat[:, 8, 512 * b : 512 * (b + 1)],
                                         start=False, stop=True,
                                         skip_group_check=True)
                    else:
                        for t in range(9):
                            nc.tensor.matmul(
                                acc.rearrange("p h w -> p (h w)"), id_sb,
                                tflat[:, t, 512 * b : 512 * (b + 1)],
                                start=(t == 0), stop=(t == 8),
                                skip_group_check=True)
                    nc.scalar.copy(
                        out16[rh][:, 8 * b : 8 * b + 8, r1, :, r2], acc)

            def emit_store(rh):
                nc.sync.dma_start(out=out[:, 32 * rh : 32 * rh + 32, :],
                                  in_=out16[rh])

            # ---------------- emission schedule ----------------
            # PE queue runs every mask matmul before the reassembly
            # accumulations; DVE queue runs row-half-0 products before
            # row-half-1 norms, so neither engine stalls on the other.
            emit_stage1(0, 8)
            emit_stage1(8, 16)
            emit_mask_block(0)
            emit_stage1(16, 24)
            emit_mask_block(1)
            emit_norm(0)
            emit_norm(1)
            emit_bounce(0)
            # row-half 1 mask matmuls fill PE while broadcasts stream
            emit_stage1(24, 32, on_act=True)
            emit_stage1(32, 34, on_act=True)
            emit_mask_block(2)
            emit_mask_block(3)
            emit_chunk(0, 0)
            emit_chunk(0, 1)
            emit_norm(2)
            emit_norm(3)
            emit_bounce(1)
            # late stage-1 copies ride ACT (off the bounce critical path)
            emit_chunk(0, 2)
            emit_chunk(0, 3)
            emit_store(0)
            for q in range(4):
                emit_chunk(1, q)
            emit_store(1)

    nc.compile()
    return nc


def _get_program():
    global _PROGRAM
    if _PROGRAM is None:
        _PROGRAM = _build_program()
    return _PROGRAM


def _shard_inputs(x, comp_w, comp_b, enc_w, enc_b):
    comp_wT = comp_w[:, :, 0, 0].T.astype(np.float16)          # [C, CC]
    comp_w2 = np.ascontiguousarray(
        np.concatenate([comp_wT, comp_wT], axis=1))            # [C, 2CC]
    # enc_w [KC, CC, 3, 3] -> tap-pair stationaries
    ew = enc_w.astype(np.float16)
    enc_wp = np.zeros((2 * CC, 3, KC), dtype=np.float16)
    enc_ws = np.zeros((CC, 3, KC), dtype=np.float16)
    for j in range(3):
        enc_wp[0:CC, j, :] = ew[:, :, 0, j].T      # di = 0 (bottom copy)
        enc_wp[CC:, j, :] = ew[:, :, 1, j].T       # di = 1 (shifted copy)
        enc_ws[:, j, :] = ew[:, :, 2, j].T         # di = 2 (single)
    # fold comp_b through the encoder taps into the encoder bias
    eb_eff = (enc_b.astype(np.float64)
              + enc_w.astype(np.float64).sum(axis=(2, 3))
              @ comp_b.astype(np.float64))
    ebv = np.ascontiguousarray(eb_eff.astype(np.float32).reshape(KC, 1))
    in_maps = []
    for core in range(NCORES):
        b, h = divmod(core, 2)
        xs = np.zeros((C, HS + 2, W), dtype=np.float32)
        lo = h * HS - 1
        s0, s1 = max(0, lo), min(H, lo + HS + 2)
        xs[:, s0 - lo : s1 - lo, :] = x[b, :, s0:s1, :]
        in_maps.append({
            "xs": np.ascontiguousarray(xs),
            "comp_w2": comp_w2,
            "enc_wp": np.ascontiguousarray(enc_wp),
            "enc_ws": np.ascontiguousarray(enc_ws),
            "enc_b": ebv,
        })
    return in_maps


def _run(inputs, trace=False):
    from concourse.bass_utils import run_bass_kernel_spmd

    nc = _get_program()
    in_maps = _shard_inputs(**inputs)
    res = run_bass_kernel_spmd(nc, in_maps, list(range(NCORES)), trace=trace)
    out = np.empty((B, C, 2 * H, 2 * W), dtype=np.float32)
    for core in range(NCORES):
        b, h = divmod(core, 2)
        out[b, :, h * 2 * HS : (h + 1) * 2 * HS, :] = \
            res.results[core]["out"].astype(np.float32)
    return out, res.exec_time_ns


def kernel(x, comp_w, comp_b, enc_w, enc_b):
    out, _ = _run(dict(x=np.asarray(x), comp_w=np.asarray(comp_w),
                       comp_b=np.asarray(comp_b), enc_w=np.asarray(enc_w),
                       enc_b=np.asarray(enc_b)))
    return out
